# revision 62
# baseline (speedup 1.0000x reference)
"""Trainium2 Bass kernel for nn_DenseGATGenerator (v3: dense-PE schedule).

Sharding: data-parallel over batch B=16 across 8 NeuronCores (2 elems/core).
All matmul operands are fp16 (1 cycle/row on the PE at any output width,
half the SBUF/HBM traffic, 2x DVE rate); accumulation, residual stream and
LN statistics stay fp32.  Measured end-to-end quantization error of this
scheme vs the fp32 reference is ~1e-3 relative (gate is 2e-2).

Scheduling for PE density (HAM stays in the warm 8/8 clock state):
  - the two batch elements are software-pipelined; every LayerNorm is
    emitted right after its element's residual update, before the other
    element's GEMM stream, so the in-order engine queues never
    head-of-line block on it.
  - ONE PSUM pool set per region (encoder / upsample+HR+decoder) instead
    of per-phase pools: no PSUM-zone reuse barriers at phase boundaries.
    The FFN f2 accumulators live in the attention-score PSUM slots; the
    decoder's bilinear accumulators do too.
  - inside attention, the score->exp->AV strands of the two elements are
    interleaved head-by-head with the qk GEMMs of the next head pair, so
    the (ACT-bound) exp chain is covered by PE work.
  - the decoder runs stage A (H W_k tiles) for both elements, then stage
    B (row-block contraction + softplus) for both, so stage-A PSUM
    evictions drain under stage-A GEMMs of the other element.
  - ACT function-table switches (exp<->gelu<->ln, ~1.3us each) are
    triggered early by dummy activations emitted while the other table's
    users are still running.

Other design points (per batch element, token-major fp32 residual stream):
  - weights consumed in natural (K, M)/(K, N) layout; LN outputs are
    transposed once per phase on the PE; qkv/f1 produce feature-major
    intermediates; proj/f2 consume them as stationary operands.
  - V is produced TOKEN-major for all heads directly from the qkv GEMM
    (x1t chunks stationary, Wv moving), so there are no per-head V
    transposes; [1 1 1 1 | v] columns give softmax row-sums from the
    same matmul as O (one batched reciprocal per head).
  - LN gains/biases folded into the following GEMM on the host; hd^-0.5
    folded into Wq; pre-decoder LN gain folded into decoder weights.
  - rstd via magic-seed Newton rsqrt on the DVE (2 iterations).
  - exp without max-subtraction (scores provably small for this family).
  - A_lr is symmetric: the transposed edge bias reuses the same A tiles,
    and (X_lr == A_lr here) the input projection reuses them too.
  - decoder symmetrization folded into weights; only upper-triangular
    column blocks of A_pred are computed and DMA'd out (fp16).
"""

import numpy as np
from contextlib import ExitStack, contextmanager

import concourse.bass as bass
import concourse.mybir as mybir
import concourse.tile as tile
from concourse import bacc
from concourse.bass_utils import run_bass_kernel_spmd
from concourse.masks import make_identity

P = 128
D = 512
DT = D // P            # 4
NLR = 256
TE = NLR // P          # 2
NHR = 512
TH = NHR // P          # 4
NH = 8
HD = 64
FF = 2048
FFT = FF // P          # 16
L = 4
KDEC = 4
BE = 2                 # batch elems per core
NCORES = 8
B = 16
EPS = 1e-5
MAGIC = 0x5F3759DF

FP32 = mybir.dt.float32
FP16 = mybir.dt.float16
I32 = mybir.dt.int32
AF = mybir.ActivationFunctionType
ALU = mybir.AluOpType


def _bcast(ap, parts=P):
    """Partition-broadcast a DRAM AP to [parts, ...] via stride-0."""
    return bass.AP(tensor=ap.tensor, offset=ap.offset, ap=[[0, parts], *ap.ap])


def build_nc(flags=(True, True, True, True, True)):
    nc = bacc.Bacc()

    x_in = nc.declare_dram_parameter("X", [BE, NLR, NLR], FP16, isOutput=False)
    ab_in = nc.declare_dram_parameter("AB", [BE, NLR, NLR], FP16,
                                      isOutput=False)
    ipW = nc.declare_dram_parameter("ipW", [NLR, D], FP16, isOutput=False)
    qkvW = nc.declare_dram_parameter("qkvW", [L, D, 3 * D], FP16,
                                     isOutput=False)
    projW = nc.declare_dram_parameter("projW", [L, D, D], FP16, isOutput=False)
    f1W = nc.declare_dram_parameter("f1W", [L, D, FF], FP16, isOutput=False)
    f2W = nc.declare_dram_parameter("f2W", [L, FF, D], FP16, isOutput=False)
    up1W = nc.declare_dram_parameter("up1W", [NLR, NHR], FP16, isOutput=False)
    up2W = nc.declare_dram_parameter("up2W", [NHR, NHR], FP16, isOutput=False)
    rqkvW = nc.declare_dram_parameter("rqkvW", [D, 3 * D], FP16,
                                      isOutput=False)
    rprojW = nc.declare_dram_parameter("rprojW", [D, D], FP16, isOutput=False)
    rf1W = nc.declare_dram_parameter("rf1W", [D, FF], FP16, isOutput=False)
    rf2W = nc.declare_dram_parameter("rf2W", [FF, D], FP16, isOutput=False)
    decW = nc.declare_dram_parameter("decW", [KDEC, D, D], FP16,
                                     isOutput=False)
    ebc = nc.declare_dram_parameter("ebc", [L, 3 * D], FP32, isOutput=False)
    epp = nc.declare_dram_parameter("epp", [L, P, 36], FP32, isOutput=False)
    gbc = nc.declare_dram_parameter("gbc", [10 * D], FP32, isOutput=False)
    gpp = nc.declare_dram_parameter("gpp", [P, 37], FP32, isOutput=False)
    out_d = nc.declare_dram_parameter("OUT", [BE, NHR, NHR], FP16,
                                      isOutput=True)

    with TileKernel(nc) as tk:
        tk.flags = flags
        tk.run(x_in, ab_in, ipW, qkvW, projW, f1W, f2W, up1W, up2W,
               rqkvW, rprojW, rf1W, rf2W, decW, ebc, epp, gbc, gpp, out_d)

    nc.finalize()
    return nc


@contextmanager
def pool_group(tc, specs):
    with ExitStack() as st:
        yield [st.enter_context(
            tc.tile_pool(name=n, bufs=b, space=sp)
        ) for n, b, sp in specs]


class TileKernel:
    def __init__(self, nc):
        self.nc = nc
        self.ctx = ExitStack()

    def __enter__(self):
        self.tc = self.ctx.enter_context(tile.TileContext(self.nc))
        return self

    def __exit__(self, *exc):
        return self.ctx.__exit__(*exc)

    def pool(self, name, bufs, space="SBUF"):
        return self.ctx.enter_context(
            self.tc.tile_pool(name=name, bufs=bufs, space=space))

    def warm_act(self, func):
        """Dummy tiny activation to trigger the ACT table switch early."""
        nc = self.nc
        nc.scalar.activation(self.warm_o[:, :], self.warm_i[:, :], func,
                             bias=1.0 if func == AF.Ln else 0.0)

    def warm_pe(self, n):
        """Keep the PE_HAM activity monitor busy across a known stall.

        Standalone fp16 LDWEIGHTS are pure PE-array activity with no
        outputs and no hazards: every real matmul self-loads its own
        stationary operand, so a stray weight load is overwritten before
        it could ever be used.  ~64 PE cycles each."""
        for _ in range(n):
            self.nc.tensor.ldweights(self.ident[:, :])

    # ---- layernorm (one elem; DVE-only rstd) -----------------------------
    def ln_b(self, src_fn, t_count, out_tile, g_ap=None, b_ap=None):
        """out[:, t, :] = (x-mean)*rstd, optionally * g + b."""
        nc = self.nc
        small = self.small
        nbt = t_count
        mvs = small.tile([P, nbt, 2], FP32, tag="ln_mvs", name="mvs")
        for t in range(t_count):
            stats = small.tile([P, 6], FP32, tag="ln_stats", name="stats")
            nc.vector.bn_stats(stats[:, :], src_fn(t))
            nc.vector.bn_aggr(mvs[:, t, :], stats[:, :])
        veps = small.tile([P, nbt], FP32, tag="ln_veps", name="veps")
        nc.vector.tensor_scalar(veps[:, :], mvs[:, :, 1], EPS, None,
                                op0=ALU.add)
        yi = small.tile([P, nbt], I32, tag="ln_yi0", name="yi")
        nc.vector.tensor_scalar(yi[:, :], veps[:, :].bitcast(I32),
                                self.one_i[:, :], None,
                                op0=ALU.arith_shift_right)
        nc.vector.tensor_tensor(yi[:, :], self.magic_i[:, 0:nbt], yi[:, :],
                                op=ALU.subtract)
        yt = small.tile([P, nbt], FP32, tag="ln_yi", name="yt")
        nc.vector.tensor_copy(yt[:, :], yi[:, :].bitcast(FP32))
        a = small.tile([P, nbt], FP32, tag="ln_a", name="a")
        for _ in range(2):
            nc.vector.tensor_tensor(a[:, :], veps[:, :], yt[:, :],
                                    op=ALU.mult)
            nc.vector.tensor_tensor(a[:, :], a[:, :], yt[:, :], op=ALU.mult)
            nc.vector.tensor_scalar(a[:, :], a[:, :], -0.5, 1.5,
                                    op0=ALU.mult, op1=ALU.add)
            nc.vector.tensor_tensor(yt[:, :], yt[:, :], a[:, :], op=ALU.mult)
        if g_ap is None:
            # evictions alternate DVE / ACT: (x-mu)*r = Id(x*r + (-mu*r))
            mr = small.tile([P, nbt], FP32, tag="ln_mr", name="mr")
            nc.vector.tensor_tensor(mr[:, :], mvs[:, :, 0], yt[:, :],
                                    op=ALU.mult)
            nc.vector.tensor_scalar(mr[:, :], mr[:, :], -1.0, None,
                                    op0=ALU.mult)
            for t in range(t_count):
                if t % 2 == 0:
                    nc.vector.tensor_scalar(
                        out_tile[:, t, :], src_fn(t), mvs[:, t, 0:1],
                        yt[:, t:t + 1],
                        op0=ALU.subtract, op1=ALU.mult)
                else:
                    nc.scalar.activation(out_tile[:, t, :], src_fn(t),
                                         AF.Identity,
                                         bias=mr[:, t:t + 1],
                                         scale=yt[:, t:t + 1])
        else:
            for t in range(t_count):
                t2 = self.mid.tile([P, D], FP32, tag="ln_t2", name="t2")
                nc.vector.tensor_scalar(
                    t2[:, :], src_fn(t), mvs[:, t, 0:1],
                    yt[:, t:t + 1],
                    op0=ALU.subtract, op1=ALU.mult)
                nc.vector.tensor_tensor(t2[:, :], t2[:, :], g_ap,
                                        op=ALU.mult)
                nc.vector.tensor_tensor(out_tile[:, t, :], t2[:, :], b_ap,
                                        op=ALU.add)

    def transpose_group(self, ps_pool, src_fn, t_count, f_count, out_tile,
                        ps_bufs=2):
        nc = self.nc
        for f in range(f_count):
            ps = ps_pool.tile([P, 2 * t_count * P], FP16, tag="o",
                              name="ps_tr", bufs=2)
            for t in range(t_count):
                nc.tensor.transpose(ps[:, t * P:(t + 1) * P], src_fn(t, f),
                                    self.ident[:, :])
            if f % 2 == 0:
                nc.scalar.copy(out_tile[:, f, :], ps[:, 0:t_count * P])
            else:
                nc.vector.tensor_copy(out_tile[:, f, :],
                                      ps[:, 0:t_count * P])

    def mm(self, ps_ap, lhs_fn, rhs_fn, k_count):
        nc = self.nc
        for k in range(k_count):
            nc.tensor.matmul(ps_ap, lhs_fn(k), rhs_fn(k),
                             start=(k == 0), stop=(k == k_count - 1))

    # ---- attention part 1 (one elem): x-transpose + token-major V --------
    def attn_p1(self, T, x1, ps, act1, act2, qkvW_sb, zero_bias, vb):
        nc = self.nc
        N = T * P
        xt = act2.tile([P, DT, N], FP16, tag="ln_t", name="x1t")
        self.transpose_group(
            ps, lambda t, f: x1[:, t, f * P:(f + 1) * P],
            T, DT, xt, ps_bufs=2)
        v_sb = act2.tile([P, T, NH, HD + 4], FP16, tag="v_sb",
                         name="v_sb", bufs=2)
        nc.vector.memset(v_sb[:, :, :, 0:4], 1.0)
        for t in range(T):
            ps_v = ps.tile([P, D], FP32, tag="mm", name="ps_v", bufs=2)
            self.mm(
                ps_v[:, :],
                lambda k, t=t: xt[:, k, t * P:(t + 1) * P],
                lambda k: qkvW_sb[:, k, 2 * D:3 * D], DT)
            src = ps_v[:, :].rearrange("p (h f) -> p h f", f=HD)
            if not zero_bias:
                nc.vector.tensor_tensor(
                    v_sb[:, t, :, 4:], src,
                    vb[:, :].rearrange("p (h f) -> p h f", f=HD),
                    op=ALU.add)
            elif t % 2 == 0:
                nc.vector.tensor_copy(v_sb[:, t, :, 4:], src)
            else:
                nc.scalar.copy(v_sb[:, t, :, 4:], src)
        o_sb = act1.tile([P, T, D], FP16, tag="o_sb", name="o_sb", bufs=2)
        return xt, v_sb, o_sb

    # ---- attention qk GEMMs for one head pair of one elem ----------------
    def attn_qk(self, T, hp, xt, ps, act2, qkvW_sb, qkvb_cols, zero_bias):
        nc = self.nc
        N = T * P
        qkv3 = act2.tile([P, 2, N], FP16, tag="qkv3", name="qkv3",
                         bufs=2)
        if T == TE and zero_bias:
            ps_qk = ps.tile([P, 2, N], FP32, tag="mm", name="ps_qk",
                            bufs=2)
            for j, mi in enumerate((hp, 4 + hp)):
                self.mm(
                    ps_qk[:, j, :],
                    lambda k, mi=mi: qkvW_sb[:, k, mi * P:(mi + 1) * P],
                    lambda k: xt[:, k, :], DT)
            nc.vector.tensor_copy(qkv3[:, :, :], ps_qk[:, :, :])
        else:
            for j, mi in enumerate((hp, 4 + hp)):
                psq = ps.tile([P, N], FP32, tag="mm", name="ps_qk",
                                bufs=2)
                self.mm(
                    psq[:, :],
                    lambda k, mi=mi: qkvW_sb[:, k, mi * P:(mi + 1) * P],
                    lambda k: xt[:, k, :], DT)
                if zero_bias:
                    if j % 2 == 0:
                        nc.vector.tensor_copy(qkv3[:, j, :], psq[:, :])
                    else:
                        nc.scalar.copy(qkv3[:, j, :], psq[:, :])
                elif j % 2 == 0:
                    nc.vector.tensor_scalar(
                        qkv3[:, j, :], psq[:, :],
                        qkvb_cols[:, mi:mi + 1], None, op0=ALU.add)
                else:
                    nc.scalar.activation(
                        qkv3[:, j, :], psq[:, :], AF.Identity,
                        bias=qkvb_cols[:, mi:mi + 1])
        return qkv3

    # ---- attention head strand (one head, one elem) ----------------------
    def attn_head(self, T, hh, h_idx, qkv3, v_sb, o_sb, ps, act1,
                  a_b, coef_cols):
        nc = self.nc
        N = T * P
        base = hh * HD
        qa = qkv3[base:base + HD, 0, :]
        ka = qkv3[base:base + HD, 1, :]
        pt = act1.tile([P, T, N], FP16, tag="pT", name="pt", bufs=2)
        if T == TE:
            ps_s = ps.tile([P, 4, N], FP32, tag="s", name="ps_s",
                           bufs=2)
            for kk in range(T):
                nc.tensor.matmul(
                    ps_s[:, kk, :],
                    ka[:, kk * P:(kk + 1) * P], qa,
                    start=True, stop=True)
            s2 = self.mid.tile([P, T, N], FP16, tag="s2", name="s2")
            nc.vector.scalar_tensor_tensor(
                s2[:, :, :], a_b[:, :, :],
                coef_cols[:, h_idx:h_idx + 1], ps_s[:, 0:T, :],
                op0=ALU.mult, op1=ALU.add)
            nc.scalar.activation(pt[:, :, :], s2[:, :, :], AF.Exp)
        else:
            for kkh in range(T // 2):
                ps_s = ps.tile([P, 2, N], FP32, tag="s", name="ps_s",
                               bufs=2)
                for kk2 in range(2):
                    kk = 2 * kkh + kk2
                    nc.tensor.matmul(
                        ps_s[:, kk2, :],
                        ka[:, kk * P:(kk + 1) * P], qa,
                        start=True, stop=True)
                nc.scalar.activation(
                    pt[:, 2 * kkh:2 * kkh + 2, :],
                    ps_s[:, :, :], AF.Exp)
        # [rowsum | o] = pT.T @ [1|v]; one batched reciprocal per head
        ps_o = ps.tile([P, T, P], FP32, tag="o", name="ps_o", bufs=2)
        for m in range(T):
            for kk in range(T):
                nc.tensor.matmul(
                    ps_o[:, m, 0:HD + 4],
                    pt[:, kk, m * P:(m + 1) * P],
                    v_sb[:, kk, h_idx, :],
                    start=(kk == 0), stop=(kk == T - 1))
        rinv = self.small.tile([P, T, 1], FP32, tag="rinv", name="rinv")
        nc.vector.reciprocal(rinv[:, :, :], ps_o[:, :, 0:1])
        for m in range(T):
            if m % 2 == 0:
                nc.vector.tensor_scalar(
                    o_sb[:, m, h_idx * HD:(h_idx + 1) * HD],
                    ps_o[:, m, 4:HD + 4], rinv[:, m, :],
                    None, op0=ALU.mult)
            else:
                nc.scalar.activation(
                    o_sb[:, m, h_idx * HD:(h_idx + 1) * HD],
                    ps_o[:, m, 4:HD + 4], AF.Copy,
                    scale=rinv[:, m, :])

    # ---- attention part 3 (one elem): o-transpose + proj + residual ------
    def attn_p3(self, T, h, o_sb, ps, act1, projW_sb, projb, zero_bias):
        nc = self.nc
        N = T * P
        ot = act1.tile([P, DT, N], FP16, tag="oT", name="ot", bufs=1)
        self.transpose_group(
            ps, lambda t, f: o_sb[:, t, f * P:(f + 1) * P],
            T, DT, ot, ps_bufs=2)
        for m in range(T):
            psp = ps.tile([P, D], FP32, tag="mm", name="ps_proj", bufs=2)
            self.mm(psp[:, :],
                    lambda k: ot[:, k, m * P:(m + 1) * P],
                    lambda k: projW_sb[:, k, :], DT)
            nc.vector.tensor_tensor(h[:, m, :], h[:, m, :], psp[:, :],
                                    op=ALU.add)
            if not zero_bias:
                nc.vector.tensor_tensor(h[:, m, :], h[:, m, :], projb,
                                        op=ALU.add)

    # ---- FFN (one elem) ---------------------------------------------------
    def ffn_body(self, T, h, x2, ps, act1, act2, f1W_sb, f2W_sb,
                 f1b_cols, f2b, zero_bias=False):
        nc = self.nc
        N = T * P
        xt = act2.tile([P, DT, N], FP16, tag="ln_t", name="x2t")
        self.transpose_group(
            ps, lambda t, f: x2[:, t, f * P:(f + 1) * P],
            T, DT, xt, ps_bufs=2)
        # f2 accumulators live in the score-tag PSUM slots (2 rows each)
        ps_f2 = [ps.tile([P, 2, D], FP32, tag="s", name=f"facc{mp}",
                         bufs=2)
                 for mp in range(T // 2)]
        half = FFT // 4
        for wave in range(4):
            gt = act1.tile([P, half, N], FP16, tag="gT", name="gt",
                           bufs=2)
            if T == TE and zero_bias:
                for jh in range(half // 2):
                    psf = ps.tile([P, 2, N], FP32, tag="mm", name="ps_f1",
                                   bufs=2)
                    for j2 in range(2):
                        mf = wave * half + 2 * jh + j2
                        self.mm(
                            psf[:, j2, :],
                            lambda k, mf=mf:
                                f1W_sb[:, k, mf * P:(mf + 1) * P],
                            lambda k: xt[:, k, :], DT)
                    mf0 = wave * half + 2 * jh
                    nc.scalar.activation(gt[:, 2 * jh:2 * jh + 2, :],
                                         psf[:, :, :], AF.Gelu,
                                         bias=f1b_cols[:, mf0:mf0 + 1])
            else:
                for j in range(half):
                    mf = wave * half + j
                    psf = ps.tile([P, N], FP32, tag="mm", name="ps_f1",
                                   bufs=2)
                    self.mm(
                        psf[:, :],
                        lambda k, mf=mf: f1W_sb[:, k, mf * P:(mf + 1) * P],
                        lambda k: xt[:, k, :], DT)
                    nc.scalar.activation(gt[:, j, :], psf[:, :], AF.Gelu,
                                         bias=f1b_cols[:, mf:mf + 1])
            for m in range(T):
                for j in range(half):
                    mf = wave * half + j
                    nc.tensor.matmul(
                        ps_f2[m // 2][:, m % 2, :],
                        gt[:, j, m * P:(m + 1) * P],
                        f2W_sb[:, mf, :],
                        start=(mf == 0), stop=(mf == FFT - 1))
        for m in range(T):
            nc.vector.tensor_tensor(h[:, m, :], h[:, m, :],
                                    ps_f2[m // 2][:, m % 2, :], op=ALU.add)
            if not zero_bias:
                nc.vector.tensor_tensor(h[:, m, :], h[:, m, :], f2b,
                                        op=ALU.add)

    # ---- model ----------------------------------------------------------
    def run(self, x_in, ab_in, ipW, qkvW, projW, f1W, f2W, up1W, up2W,
            rqkvW, rprojW, rf1W, rf2W, decW, ebc, epp, gbc, gpp, out_d):
        nc = self.nc
        tc = self.tc
        triv_ip, triv_encn, zb_enc, zb_hr, reuse_x = self.flags

        const = self.pool("const", 1)
        persist = self.pool("persist", 1)
        self.small = self.pool("small", 4)
        self.mid = self.pool("mid", 2)

        ident32 = const.tile([P, P], FP32)
        make_identity(nc, ident32[:, :])
        self.ident = const.tile([P, P], FP16)
        nc.vector.tensor_copy(self.ident[:, :], ident32[:, :])
        self.one_i = const.tile([P, 1], I32)
        nc.vector.memset(self.one_i[:, :], 1)
        self.magic_i = const.tile([P, TH], I32)
        nc.vector.memset(self.magic_i[:, :], MAGIC)
        self.warm_i = const.tile([P, 1], FP32)
        nc.vector.memset(self.warm_i[:, :], 0.0)
        self.warm_o = const.tile([P, 1], FP32)

        gpp_sb = persist.tile([P, 37], FP32)
        nc.sync.dma_start(out=gpp_sb[:, :], in_=gpp[:, :])

        hr_res = self.pool("hr_res", 1)
        h_hr = [hr_res.tile([P, TH, D], FP32, tag=f"Hhr{b}", name=f"Hhr{b}")
                for b in range(BE)]
        up_w = self.pool("up_w", 1)
        up_hf = self.pool("up_hf", 1)
        hfs = [up_hf.tile([P, TE, D], FP16, tag=f"hf{b}", name=f"hf{b}")
               for b in range(BE)]

        # ================= encoder region =================
        with pool_group(tc, [("enc_res", 1, "SBUF"), ("enc_a1", 1, "SBUF"),
                             ("enc_a2", 2, "SBUF"), ("enc_w", 1, "SBUF"),
                             ("enc_pk", 1, "SBUF"),
                             ("ps1", 1, "PSUM")]) as \
                (enc_res, act1, act2, enc_w, enc_pk, ps):
            h_enc = [enc_res.tile([P, TE, D], FP32, tag=f"Henc{b}",
                                  name=f"Henc{b}") for b in range(BE)]
            a_t = [enc_res.tile([P, TE, NLR], FP16, tag=f"A{b}", name=f"A{b}")
                   for b in range(BE)]
            for b in range(BE):
                for t in range(TE):
                    nc.scalar.dma_start(
                        out=a_t[b][:, t, :],
                        in_=ab_in[b].rearrange("(t p) m -> p t m",
                                               p=P)[:, t, :])

            def load_enc(l):
                w = {}
                w["qkv"] = enc_w.tile([P, DT, 3 * D], FP16, tag="qkvW",
                                      name="qkvW_sb", bufs=2)
                nc.sync.dma_start(
                    out=w["qkv"][:, :, :],
                    in_=qkvW[l].rearrange("(k p) n -> p k n", p=P))
                w["proj"] = enc_w.tile([P, DT, D], FP16, tag="projW",
                                       name="projW_sb", bufs=1)
                nc.sync.dma_start(
                    out=w["proj"][:, :, :],
                    in_=projW[l].rearrange("(k p) n -> p k n", p=P))
                w["f1"] = enc_w.tile([P, DT, FF], FP16, tag="f1W",
                                     name="f1W_sb", bufs=1)
                nc.sync.dma_start(
                    out=w["f1"][:, :, :],
                    in_=f1W[l].rearrange("(k p) n -> p k n", p=P))
                w["f2"] = enc_w.tile([P, FFT, D], FP16, tag="f2W",
                                     name="f2W_sb", bufs=1)
                nc.sync.dma_start(
                    out=w["f2"][:, :, :],
                    in_=f2W[l].rearrange("(k p) n -> p k n", p=P))
                w["ebc"] = enc_pk.tile([P, 3, D], FP32, tag="ebc",
                                       name="ebc_sb", bufs=1)
                nc.sync.dma_start(
                    out=w["ebc"][:, :, :],
                    in_=_bcast(ebc[l].rearrange("(a b) -> a b", b=D)))
                w["epp"] = enc_pk.tile([P, 36], FP32, tag="epp",
                                       name="epp_sb", bufs=2)
                nc.sync.dma_start(out=w["epp"][:, :], in_=epp[l])
                return w

            cur = load_enc(0)

            # ---------------- phase 0: input projection ----------------
            with pool_group(tc, [("ip_sb", 1, "SBUF"),
                                 ("ip_w", 1, "SBUF")]) as (ip_sb, ip_w):
                gbc_ip = ip_w.tile([P, 3, D], FP32)
                nc.scalar.dma_start(
                    out=gbc_ip[:, :, :],
                    in_=_bcast(gbc[0:3 * D].rearrange("(a b) -> a b", b=D)))
                ipW_sb = ip_w.tile([P, TE, D], FP16)
                nc.scalar.dma_start(
                    out=ipW_sb[:, :, :],
                    in_=ipW[:, :].rearrange("(k p) n -> p k n", p=P))
                x_sbs = []
                for b in range(BE):
                    if reuse_x:   # X_lr == sym(A_lr): reuse the A tiles
                        x_sbs.append(a_t[b])
                    else:
                        x_sb = ip_sb.tile([P, TE, NLR], FP16, tag=f"x{b}",
                                          name=f"x{b}")
                        nc.scalar.dma_start(
                            out=x_sb[:, :, :],
                            in_=x_in[b].rearrange("(t p) m -> p t m", p=P))
                        x_sbs.append(x_sb)
                # both elems' transposes+GEMMs first (b0's PSUM in "mm"
                # slots, b1's in an "s" slot) so neither waits on the
                # other's LN; then the LN/gelu chains.
                src_fns = []
                for b in range(BE):
                    xt = ip_sb.tile([P, TE, NLR], FP16, tag="xt", name="xt",
                                    bufs=2)
                    self.transpose_group(
                        ps,
                        lambda t, f, b=b: x_sbs[b][:, t, f * P:(f + 1) * P],
                        TE, TE, xt)
                    if b == 0:
                        zts = []
                        for m in range(TE):
                            zps = ps.tile([P, D], FP32, tag="mm", name="zps",
                                          bufs=2)
                            self.mm(zps[:, :],
                                    lambda k: xt[:, k, m * P:(m + 1) * P],
                                    lambda k: ipW_sb[:, k, :], TE)
                            zts.append(zps)
                        src_fns.append(lambda t, zts=zts: zts[t][:, :])
                    else:
                        zs = ps.tile([P, TE, D], FP32, tag="s", name="zps1",
                                     bufs=2)
                        for m in range(TE):
                            self.mm(zs[:, m, :],
                                    lambda k: xt[:, k, m * P:(m + 1) * P],
                                    lambda k: ipW_sb[:, k, :], TE)
                        src_fns.append(lambda t, zs=zs: zs[:, t, :])
                for b in range(BE):
                    src_fn = src_fns[b]
                    if not triv_ip:
                        z2 = ip_sb.tile([P, TE, D], FP32, tag=f"z{b}",
                                        name=f"z{b}")
                        for m in range(TE):
                            nc.vector.tensor_tensor(z2[:, m, :],
                                                    src_fn(m),
                                                    gbc_ip[:, 0, :],
                                                    op=ALU.add)
                        src_fn = lambda t, z2=z2: z2[:, t, :]
                    lno = ip_sb.tile([P, TE, D], FP32, tag=f"lnout{b}",
                                     name=f"lnout{b}")
                    self.ln_b(src_fn, TE, lno,
                              None if triv_ip else gbc_ip[:, 1, :],
                              None if triv_ip else gbc_ip[:, 2, :])
                    for t in range(TE):
                        nc.scalar.activation(h_enc[b][:, t, :],
                                             lno[:, t, :], AF.Gelu)
                self.warm_act(AF.Exp)

            # ---------------- encoder layers (pipelined) ----------------
            xs = [None, None]
            for b in range(BE):
                xs[b] = act2.tile([P, TE, D], FP16, tag="ln_out",
                                  name=f"xln{b}", bufs=2)
                self.ln_b(lambda t, b=b: h_enc[b][:, t, :], TE, xs[b])

            for l in range(L):
                w = cur
                if l + 1 < L:
                    cur = load_enc(l + 1)
                if l == L - 2:
                    up1W_sb = up_w.tile([P, TE, NHR], FP16)
                    nc.sync.dma_start(
                        out=up1W_sb[:, :, :],
                        in_=up1W[:, :].rearrange("(k p) n -> p k n", p=P))
                    up2W_sb = up_w.tile([P, TH, NHR], FP16)
                    nc.sync.dma_start(
                        out=up2W_sb[:, :, :],
                        in_=up2W[:, :].rearrange("(k p) n -> p k n", p=P))
                    if not triv_encn:
                        gbc_en = up_w.tile([P, 2, D], FP32)
                        nc.sync.dma_start(
                            out=gbc_en[:, :, :],
                            in_=_bcast(gbc[3 * D:5 * D].rearrange(
                                "(a b) -> a b", b=D)))

                st = [self.attn_p1(TE, xs[b], ps, act1, act2, w["qkv"],
                                   zb_enc, w["ebc"][:, 2, :])
                      for b in range(BE)]
                for hp in range(NH // 2):
                    qk = [self.attn_qk(TE, hp, st[b][0], ps, act2,
                                       w["qkv"], w["epp"][:, 0:12], zb_enc)
                          for b in range(BE)]
                    for hh in range(2):
                        for b in range(BE):
                            self.attn_head(
                                TE, hh, 2 * hp + hh, qk[b], st[b][1],
                                st[b][2], ps, act1, a_t[b],
                                w["epp"][:, 28:36])
                for b in range(BE):
                    self.attn_p3(TE, h_enc[b], st[b][2], ps, act1,
                                 w["proj"], w["ebc"][:, 0, :], zb_enc)
                    if b == 0:
                        self.warm_act(AF.Gelu)
                    xs[b] = act2.tile([P, TE, D], FP16, tag="ln_out",
                                      name=f"xln{b}", bufs=2)
                    self.ln_b(lambda t, b=b: h_enc[b][:, t, :], TE, xs[b])
                for b in range(BE):
                    self.ffn_body(
                        TE, h_enc[b], xs[b], ps, act1, act2,
                        w["f1"], w["f2"], w["epp"][:, 12:28],
                        w["ebc"][:, 1, :], zero_bias=zb_enc)
                    if b == 0 and l + 1 < L:
                        self.warm_act(AF.Exp)
                    if l + 1 < L:
                        xs[b] = act2.tile([P, TE, D], FP16, tag="ln_out",
                                          name=f"xln{b}", bufs=2)
                        self.ln_b(lambda t, b=b: h_enc[b][:, t, :], TE,
                                  xs[b])
                    else:
                        self.ln_b(
                            lambda t, b=b: h_enc[b][:, t, :], TE, hfs[b],
                            None if triv_encn else gbc_en[:, 0, :],
                            None if triv_encn else gbc_en[:, 1, :])

        # ================= upsample + HR + decoder region =================
        with pool_group(tc, [("hr_a1", 1, "SBUF"), ("hr_a2", 2, "SBUF"),
                             ("hr_w2", 1, "SBUF"), ("dec_sb", 1, "SBUF"),
                             ("dec_sb2", 2, "SBUF"),
                             ("ps2", 1, "PSUM")]) as \
                (act1, act2, hr_w2, dec_sb, dec_sb2, ps):
            rqkvW_sb = hr_w2.tile([P, DT, 3 * D], FP16, tag="qkvW")
            nc.sync.dma_start(
                out=rqkvW_sb[:, :, :],
                in_=rqkvW[:, :].rearrange("(k p) n -> p k n", p=P))
            gbc_hr = hr_w2.tile([P, 2, D], FP32, tag="gbc_hr")
            nc.sync.dma_start(
                out=gbc_hr[:, :, :],
                in_=_bcast(gbc[5 * D:7 * D].rearrange("(a b) -> a b", b=D)))
            if not zb_hr:
                vb_hr = hr_w2.tile([P, 1, D], FP32, tag="vb_hr")
                nc.sync.dma_start(
                    out=vb_hr[:, :, :],
                    in_=_bcast(gbc[9 * D:10 * D].rearrange(
                        "(a b) -> a b", b=D)))
            rprojW_sb = hr_w2.tile([P, DT, D], FP16, tag="projW")
            nc.sync.dma_start(
                out=rprojW_sb[:, :, :],
                in_=rprojW[:, :].rearrange("(k p) n -> p k n", p=P))
            rf1W_sb = hr_w2.tile([P, DT, FF], FP16, tag="f1W")
            nc.sync.dma_start(
                out=rf1W_sb[:, :, :],
                in_=rf1W[:, :].rearrange("(k p) n -> p k n", p=P))
            rf2W_sb = hr_w2.tile([P, FFT, D], FP16, tag="f2W")
            nc.sync.dma_start(
                out=rf2W_sb[:, :, :],
                in_=rf2W[:, :].rearrange("(k p) n -> p k n", p=P))
            decW_sb = hr_w2.tile([P, KDEC, DT, D], FP16, tag="decW")
            nc.sync.dma_start(
                out=decW_sb[:, :, :, :],
                in_=decW[:, :, :].rearrange("kd (k p) m -> p kd k m", p=P))

            rx = [None, None]
            for b in range(BE):
                g1 = act1.tile([P, TH, D], FP16, tag="g1", name="g1")
                for mh in range(TH):
                    psu = ps.tile([P, D], FP32, tag="mm", name="ps_up",
                             bufs=2)
                    self.mm(psu[:, :],
                            lambda k: up1W_sb[:, k, mh * P:(mh + 1) * P],
                            lambda k: hfs[b][:, k, :], TE)
                    nc.scalar.activation(g1[:, mh, :], psu[:, :], AF.Gelu,
                                         bias=gpp_sb[:, mh:mh + 1])
                for mh in range(TH):
                    psu = ps.tile([P, D], FP32, tag="mm", name="ps_up",
                             bufs=2)
                    self.mm(psu[:, :],
                            lambda k: up2W_sb[:, k, mh * P:(mh + 1) * P],
                            lambda k: g1[:, k, :], TH)
                    nc.vector.tensor_scalar(
                        h_hr[b][:, mh, :], psu[:, :],
                        gpp_sb[:, 4 + mh:5 + mh], None, op0=ALU.add)
                if b == 0:
                    self.warm_act(AF.Exp)
                rx[b] = act2.tile([P, TH, D], FP16, tag="ln_out",
                                  name=f"rx{b}", bufs=2)
                self.ln_b(lambda t, b=b: h_hr[b][:, t, :], TH, rx[b])

            st = [self.attn_p1(TH, rx[b], ps, act1, act2, rqkvW_sb,
                               zb_hr, None if zb_hr else vb_hr[:, 0, :])
                  for b in range(BE)]
            for hp in range(NH // 2):
                qk = [self.attn_qk(TH, hp, st[b][0], ps, act2,
                                   rqkvW_sb, gpp_sb[:, 8:20], zb_hr)
                      for b in range(BE)]
                for hh in range(2):
                    for b in range(BE):
                        self.attn_head(
                            TH, hh, 2 * hp + hh, qk[b], st[b][1],
                            st[b][2], ps, act1, None, None)
            for b in range(BE):
                self.attn_p3(TH, h_hr[b], st[b][2], ps, act1,
                             rprojW_sb, gbc_hr[:, 0, :], zb_hr)
                if b == 0:
                    self.warm_act(AF.Gelu)
                rx[b] = act2.tile([P, TH, D], FP16, tag="ln_out",
                                  name=f"rx{b}", bufs=2)
                self.ln_b(lambda t, b=b: h_hr[b][:, t, :], TH, rx[b])
            hf2s = [None, None]
            for b in range(BE):
                self.ffn_body(
                    TH, h_hr[b], rx[b], ps, act1, act2,
                    rf1W_sb, rf2W_sb, gpp_sb[:, 20:36],
                    gbc_hr[:, 1, :], zero_bias=zb_hr)
                if b == 0:
                    self.warm_act(AF.Ln)
                hf2s[b] = dec_sb2.tile([P, TH, D], FP16, tag="hf2",
                                       name=f"hf2{b}")
                self.ln_b(lambda t, b=b: h_hr[b][:, t, :], TH, hf2s[b])

            # ---------------- decoder ----------------
            # only one kd-slice of M1 = H W_kd^T is materialized at a time;
            # the bilinear row-block accumulators (2 PSUM "s" slots = 4 row
            # blocks) integrate over kd, and m1(kd+1) GEMMs cover ak(kd)'s
            # eviction latency.
            for b in range(BE):
                hft = dec_sb.tile([P, DT, NHR], FP16, tag="hft",
                                  name="hft", bufs=2)
                self.transpose_group(
                    ps, lambda t, f: hf2s[b][:, t, f * P:(f + 1) * P],
                    TH, DT, hft)
                psa = [ps.tile([P, 2, NHR], FP32, tag="s", name="ps_ak",
                               bufs=2) for _ in range(TH // 2)]
                m1ts = []

                def dec_m1(kd, hft=hft, b=b):
                    m1t = dec_sb.tile([P, DT, NHR], FP16, tag="m1t",
                                      name="m1t", bufs=2)
                    for mi in range(DT):
                        psd = ps.tile([P, NHR], FP32, tag="mm", name="ps_m1",
                                      bufs=2)
                        self.mm(
                            psd[:, :],
                            lambda k, kd=kd, mi=mi:
                                decW_sb[:, kd, k, mi * P:(mi + 1) * P],
                            lambda k: hft[:, k, :], DT)
                        if mi % 2 == 0:
                            nc.vector.tensor_copy(m1t[:, mi, :], psd[:, :])
                        else:
                            nc.scalar.copy(m1t[:, mi, :], psd[:, :])
                    return m1t

                def dec_ak(kd, m1t, hft=hft):
                    for md in range(TH):
                        wcol = NHR - md * P   # upper-tri column blocks
                        for k in range(DT):
                            nc.tensor.matmul(
                                psa[md // 2][:, md % 2, 0:wcol],
                                m1t[:, k, md * P:(md + 1) * P],
                                hft[:, k, md * P:],
                                start=(kd == 0 and k == 0),
                                stop=(kd == KDEC - 1 and k == DT - 1))

                m1ts.append(dec_m1(0))
                for kd in range(1, KDEC):
                    m1ts.append(dec_m1(kd))
                    dec_ak(kd - 1, m1ts[kd - 1])
                dec_ak(KDEC - 1, m1ts[KDEC - 1])
                out_sb = dec_sb2.tile([P, TH, NHR], FP16, tag="out",
                                      name="out_sb", bufs=1)
                for md in range(TH):
                    wcol = NHR - md * P
                    # softplus(x/K + b) = ln(1 + exp(x/K + b))
                    sp_e = self.mid.tile([P, NHR], FP32, tag="sp_e",
                                         name="sp_e")
                    nc.scalar.activation(sp_e[:, 0:wcol],
                                         psa[md // 2][:, md % 2, 0:wcol],
                                         AF.Exp,
                                         bias=gpp_sb[:, 36:37],
                                         scale=1.0 / KDEC)
                    nc.scalar.activation(out_sb[:, md, md * P:],
                                         sp_e[:, 0:wcol],
                                         AF.Ln, bias=1.0)
                    nc.sync.dma_start(
                        out=out_d[b].rearrange("(t p) m -> p t m",
                                               p=P)[:, md, md * P:],
                        in_=out_sb[:, md, md * P:])


# --------------------------------------------------------------------------
# host-side driver
# --------------------------------------------------------------------------
_CACHE = {}
_TRIU = np.triu_indices(NHR, k=1)


def _np(x):
    return np.ascontiguousarray(np.asarray(x, dtype=np.float32))


def _h(x):
    return np.ascontiguousarray(np.asarray(x).astype(np.float16))


def kernel(**inputs):
    res = run_on_device(inputs)
    full = np.concatenate([res.results[c]["OUT"] for c in range(NCORES)],
                          axis=0)  # (16, 512, 512)
    return np.ascontiguousarray(full[:, _TRIU[0], _TRIU[1]]).astype(np.float32)


def _fold_ln(g, b, w, bias):
    """(xn*g + b) @ w + bias  ==  xn @ (diag(g)w) + (bias + b @ w)."""
    w64 = w.astype(np.float64)
    w2 = (g.astype(np.float64)[:, None] * w64).astype(np.float32)
    b2 = (bias.astype(np.float64) + b.astype(np.float64) @ w64).astype(
        np.float32)
    return w2, b2


def run_on_device(inputs, **run_kwargs):
    inp = {k: _np(v) for k, v in inputs.items()}

    qkvW_f = np.empty_like(inp["e_qkvW"])
    qkvb_f = np.empty_like(inp["e_qkvb"])
    f1W_f = np.empty_like(inp["e_f1W"])
    f1b_f = np.empty_like(inp["e_f1b"])
    for l in range(L):
        qkvW_f[l], qkvb_f[l] = _fold_ln(inp["e_n1g"][l], inp["e_n1b"][l],
                                        inp["e_qkvW"][l], inp["e_qkvb"][l])
        f1W_f[l], f1b_f[l] = _fold_ln(inp["e_n2g"][l], inp["e_n2b"][l],
                                      inp["e_f1W"][l], inp["e_f1b"][l])
    rqkvW_f, rqkvb_f = _fold_ln(inp["r_n1g"], inp["r_n1b"],
                                inp["r_qkvW"], inp["r_qkvb"])
    rf1W_f, rf1b_f = _fold_ln(inp["r_n2g"], inp["r_n2b"],
                              inp["r_f1W"], inp["r_f1b"])
    # fold hd^-0.5 into the q columns (weights AND biases)
    qkvW_f[:, :, 0:D] *= HD ** -0.5
    qkvb_f[:, 0:D] *= HD ** -0.5
    rqkvW_f[:, 0:D] *= HD ** -0.5
    rqkvb_f[0:D] *= HD ** -0.5
    # fold the pre-decoder LN gain into the (symmetrized) decoder weights;
    # its bias is zero for this model (asserted).
    assert np.abs(inp["hrn_b"]).max() == 0.0, "hrn_b fold requires zero bias"
    dec_sym = 0.5 * (inp["dec_W"] + inp["dec_W"].transpose(0, 2, 1))
    g = inp["hrn_g"].astype(np.float64)
    dec_sym = (g[None, :, None] * dec_sym.astype(np.float64)
               * g[None, None, :]).astype(np.float32)

    # the transposed-score path uses A^T == A; guarantee symmetry
    a_sym = 0.5 * (inp["A_lr"] + inp["A_lr"].transpose(0, 2, 1))

    flags = (
        bool((inp["ip_g"] == 1).all() and (inp["ip_bt"] == 0).all()
             and (inp["ip_b"] == 0).all()),
        bool((inp["encn_g"] == 1).all() and (inp["encn_b"] == 0).all()),
        bool((inp["e_projb"] == 0).all() and (inp["e_f2b"] == 0).all()
             and (qkvb_f == 0).all() and (f1b_f == 0).all()),
        bool((inp["r_projb"] == 0).all() and (inp["r_f2b"] == 0).all()
             and (rqkvb_f == 0).all() and (rf1b_f == 0).all()),
        bool(np.array_equal(inp["X_lr"], a_sym)),
    )
    if _CACHE.get("flags") != flags:
        _CACHE["nc"] = build_nc(flags)
        _CACHE["flags"] = flags
    nc = _CACHE["nc"]

    ebc = np.stack([
        np.concatenate([inp["e_projb"][l], inp["e_f2b"][l],
                        qkvb_f[l, 2 * D:3 * D]])
        for l in range(L)
    ])
    epp = np.stack([
        np.concatenate([
            qkvb_f[l].reshape(12, P).T,
            f1b_f[l].reshape(FFT, P).T,
            np.broadcast_to(inp["e_ebs"][l] * inp["e_ebW"][l], (P, NH)),
        ], axis=1)
        for l in range(L)
    ])
    gbc = np.concatenate([
        inp["ip_b"], inp["ip_g"], inp["ip_bt"], inp["encn_g"], inp["encn_b"],
        inp["r_projb"], inp["r_f2b"], inp["hrn_g"], inp["hrn_b"],
        rqkvb_f[2 * D:3 * D],
    ])
    gpp = np.concatenate([
        inp["up1b"].reshape(TH, P).T,
        inp["up2b"].reshape(TH, P).T,
        rqkvb_f.reshape(12, P).T,
        rf1b_f.reshape(FFT, P).T,
        np.broadcast_to(inp["dec_b"][0], (P, 1)),
    ], axis=1)

    shared = {
        "ipW": _h(inp["ip_W"]), "qkvW": _h(qkvW_f), "projW": _h(inp["e_projW"]),
        "f1W": _h(f1W_f), "f2W": _h(inp["e_f2W"]), "up1W": _h(inp["up1W"]),
        "up2W": _h(inp["up2W"]), "rqkvW": _h(rqkvW_f),
        "rprojW": _h(inp["r_projW"]),
        "rf1W": _h(rf1W_f), "rf2W": _h(inp["r_f2W"]),
        "decW": _h(dec_sym),
        "ebc": np.ascontiguousarray(ebc), "epp": np.ascontiguousarray(epp),
        "gbc": np.ascontiguousarray(gbc), "gpp": np.ascontiguousarray(gpp),
    }
    in_maps = []
    for c in range(NCORES):
        m = dict(shared)
        m["X"] = _h(inp["X_lr"][c * BE:(c + 1) * BE])
        m["AB"] = _h(a_sym[c * BE:(c + 1) * BE])
        in_maps.append(m)

    return run_bass_kernel_spmd(nc, in_maps, list(range(NCORES)), **run_kwargs)


if __name__ == "__main__":
    import time
    t0 = time.time()
    nc = build_nc()
    print(f"build+finalize: {time.time() - t0:.1f}s, insts={len(nc.inst_map)}")


# revision 64
# speedup vs baseline: 1.0288x; 1.0288x over previous
"""Trainium2 Bass kernel for nn_DenseGATGenerator (v3: dense-PE schedule).

Sharding: data-parallel over batch B=16 across 8 NeuronCores (2 elems/core).
All matmul operands are fp16 (1 cycle/row on the PE at any output width,
half the SBUF/HBM traffic, 2x DVE rate); accumulation, residual stream and
LN statistics stay fp32.  Measured end-to-end quantization error of this
scheme vs the fp32 reference is ~1e-3 relative (gate is 2e-2).

Scheduling for PE density (HAM stays in the warm 8/8 clock state):
  - the two batch elements are software-pipelined; every LayerNorm is
    emitted right after its element's residual update, before the other
    element's GEMM stream, so the in-order engine queues never
    head-of-line block on it.
  - ONE PSUM pool set per region (encoder / upsample+HR+decoder) instead
    of per-phase pools: no PSUM-zone reuse barriers at phase boundaries.
    The FFN f2 accumulators live in the attention-score PSUM slots; the
    decoder's bilinear accumulators do too.
  - inside attention, the score->exp->AV strands of the two elements are
    interleaved head-by-head with the qk GEMMs of the next head pair, so
    the (ACT-bound) exp chain is covered by PE work.
  - the decoder runs stage A (H W_k tiles) for both elements, then stage
    B (row-block contraction + softplus) for both, so stage-A PSUM
    evictions drain under stage-A GEMMs of the other element.
  - ACT function-table switches (exp<->gelu<->ln, ~1.3us each) are
    triggered early by dummy activations emitted while the other table's
    users are still running.

Other design points (per batch element, token-major fp32 residual stream):
  - weights consumed in natural (K, M)/(K, N) layout; LN outputs are
    transposed once per phase on the PE; qkv/f1 produce feature-major
    intermediates; proj/f2 consume them as stationary operands.
  - V is produced TOKEN-major for all heads directly from the qkv GEMM
    (x1t chunks stationary, Wv moving), so there are no per-head V
    transposes; [1 1 1 1 | v] columns give softmax row-sums from the
    same matmul as O (one batched reciprocal per head).
  - LN gains/biases folded into the following GEMM on the host; hd^-0.5
    folded into Wq; pre-decoder LN gain folded into decoder weights.
  - rstd via magic-seed Newton rsqrt on the DVE (2 iterations).
  - exp without max-subtraction (scores provably small for this family).
  - A_lr is symmetric: the transposed edge bias reuses the same A tiles,
    and (X_lr == A_lr here) the input projection reuses them too.
  - decoder symmetrization folded into weights; only upper-triangular
    column blocks of A_pred are computed and DMA'd out (fp16).
"""

import numpy as np
from contextlib import ExitStack, contextmanager

import concourse.bass as bass
import concourse.mybir as mybir
import concourse.tile as tile
from concourse import bacc
from concourse.bass_utils import run_bass_kernel_spmd
from concourse.masks import make_identity

P = 128
D = 512
DT = D // P            # 4
NLR = 256
TE = NLR // P          # 2
NHR = 512
TH = NHR // P          # 4
NH = 8
HD = 64
FF = 2048
FFT = FF // P          # 16
L = 4
KDEC = 4
BE = 2                 # batch elems per core
NCORES = 8
B = 16
EPS = 1e-5
MAGIC = 0x5F3759DF

FP32 = mybir.dt.float32
FP16 = mybir.dt.float16
I32 = mybir.dt.int32
AF = mybir.ActivationFunctionType
ALU = mybir.AluOpType


def _bcast(ap, parts=P):
    """Partition-broadcast a DRAM AP to [parts, ...] via stride-0."""
    return bass.AP(tensor=ap.tensor, offset=ap.offset, ap=[[0, parts], *ap.ap])


def build_nc(flags=(True, True, True, True, True)):
    nc = bacc.Bacc()

    x_in = nc.declare_dram_parameter("X", [BE, NLR, NLR], FP16, isOutput=False)
    ab_in = nc.declare_dram_parameter("AB", [BE, NLR, NLR], FP16,
                                      isOutput=False)
    ipW = nc.declare_dram_parameter("ipW", [NLR, D], FP16, isOutput=False)
    qkvW = nc.declare_dram_parameter("qkvW", [L, D, 3 * D], FP16,
                                     isOutput=False)
    projW = nc.declare_dram_parameter("projW", [L, D, D], FP16, isOutput=False)
    f1W = nc.declare_dram_parameter("f1W", [L, D, FF], FP16, isOutput=False)
    f2W = nc.declare_dram_parameter("f2W", [L, FF, D], FP16, isOutput=False)
    up1W = nc.declare_dram_parameter("up1W", [NLR, NHR], FP16, isOutput=False)
    up2W = nc.declare_dram_parameter("up2W", [NHR, NHR], FP16, isOutput=False)
    rqkvW = nc.declare_dram_parameter("rqkvW", [D, 3 * D], FP16,
                                      isOutput=False)
    rprojW = nc.declare_dram_parameter("rprojW", [D, D], FP16, isOutput=False)
    rf1W = nc.declare_dram_parameter("rf1W", [D, FF], FP16, isOutput=False)
    rf2W = nc.declare_dram_parameter("rf2W", [FF, D], FP16, isOutput=False)
    decW = nc.declare_dram_parameter("decW", [KDEC, D, D], FP16,
                                     isOutput=False)
    ebc = nc.declare_dram_parameter("ebc", [L, 3 * D], FP32, isOutput=False)
    epp = nc.declare_dram_parameter("epp", [L, P, 36], FP32, isOutput=False)
    gbc = nc.declare_dram_parameter("gbc", [10 * D], FP32, isOutput=False)
    gpp = nc.declare_dram_parameter("gpp", [P, 37], FP32, isOutput=False)
    out_d = nc.declare_dram_parameter("OUT", [BE, NHR, NHR], FP16,
                                      isOutput=True)

    with TileKernel(nc) as tk:
        tk.flags = flags
        tk.run(x_in, ab_in, ipW, qkvW, projW, f1W, f2W, up1W, up2W,
               rqkvW, rprojW, rf1W, rf2W, decW, ebc, epp, gbc, gpp, out_d)

    nc.finalize()
    return nc


@contextmanager
def pool_group(tc, specs):
    with ExitStack() as st:
        yield [st.enter_context(
            tc.tile_pool(name=n, bufs=b, space=sp)
        ) for n, b, sp in specs]


class TileKernel:
    def __init__(self, nc):
        self.nc = nc
        self.ctx = ExitStack()

    def __enter__(self):
        self.tc = self.ctx.enter_context(tile.TileContext(self.nc))
        return self

    def __exit__(self, *exc):
        return self.ctx.__exit__(*exc)

    def pool(self, name, bufs, space="SBUF"):
        return self.ctx.enter_context(
            self.tc.tile_pool(name=name, bufs=bufs, space=space))

    def warm_act(self, func):
        """Dummy tiny activation to trigger the ACT table switch early."""
        nc = self.nc
        nc.scalar.activation(self.warm_o[:, :], self.warm_i[:, :], func,
                             bias=1.0 if func == AF.Ln else 0.0)

    def warm_pe(self, n):
        """Keep the PE_HAM activity monitor busy across a known stall.

        Standalone fp16 LDWEIGHTS are pure PE-array activity with no
        outputs and no hazards: every real matmul self-loads its own
        stationary operand, so a stray weight load is overwritten before
        it could ever be used.  ~64 PE cycles each."""
        for _ in range(n):
            self.nc.tensor.ldweights(self.ident[:, :])

    # ---- layernorm (one elem; DVE-only rstd) -----------------------------
    def ln_b(self, src_fn, t_count, out_tile, g_ap=None, b_ap=None):
        """out[:, t, :] = (x-mean)*rstd, optionally * g + b."""
        nc = self.nc
        small = self.small
        nbt = t_count
        mvs = small.tile([P, nbt, 2], FP32, tag="ln_mvs", name="mvs")
        for t in range(t_count):
            stats = small.tile([P, 6], FP32, tag="ln_stats", name="stats")
            nc.vector.bn_stats(stats[:, :], src_fn(t))
            nc.vector.bn_aggr(mvs[:, t, :], stats[:, :])
        veps = small.tile([P, nbt], FP32, tag="ln_veps", name="veps")
        nc.vector.tensor_scalar(veps[:, :], mvs[:, :, 1], EPS, None,
                                op0=ALU.add)
        yi = small.tile([P, nbt], I32, tag="ln_yi0", name="yi")
        nc.vector.tensor_scalar(yi[:, :], veps[:, :].bitcast(I32),
                                self.one_i[:, :], None,
                                op0=ALU.arith_shift_right)
        nc.vector.tensor_tensor(yi[:, :], self.magic_i[:, 0:nbt], yi[:, :],
                                op=ALU.subtract)
        # one Newton step off the magic seed (rel err <= ~0.2%, far inside
        # the tolerance); the seed is used via a bitcast view (no copy).
        ysd = yi[:, :].bitcast(FP32)
        yt = small.tile([P, nbt], FP32, tag="ln_yi", name="yt")
        a = small.tile([P, nbt], FP32, tag="ln_a", name="a")
        nc.vector.tensor_tensor(a[:, :], veps[:, :], ysd, op=ALU.mult)
        nc.vector.tensor_tensor(a[:, :], a[:, :], ysd, op=ALU.mult)
        nc.vector.tensor_scalar(a[:, :], a[:, :], -0.5, 1.5,
                                op0=ALU.mult, op1=ALU.add)
        nc.vector.tensor_tensor(yt[:, :], ysd, a[:, :], op=ALU.mult)
        if g_ap is None:
            # evictions alternate DVE / ACT: (x-mu)*r = Id(x*r + (-mu*r))
            mr = small.tile([P, nbt], FP32, tag="ln_mr", name="mr")
            nc.vector.tensor_tensor(mr[:, :], mvs[:, :, 0], yt[:, :],
                                    op=ALU.mult)
            nc.vector.tensor_scalar(mr[:, :], mr[:, :], -1.0, None,
                                    op0=ALU.mult)
            for t in range(t_count):
                if t % 2 == 0:
                    nc.vector.tensor_scalar(
                        out_tile[:, t, :], src_fn(t), mvs[:, t, 0:1],
                        yt[:, t:t + 1],
                        op0=ALU.subtract, op1=ALU.mult)
                else:
                    nc.scalar.activation(out_tile[:, t, :], src_fn(t),
                                         AF.Identity,
                                         bias=mr[:, t:t + 1],
                                         scale=yt[:, t:t + 1])
        else:
            for t in range(t_count):
                t2 = self.mid.tile([P, D], FP32, tag="ln_t2", name="t2")
                nc.vector.tensor_scalar(
                    t2[:, :], src_fn(t), mvs[:, t, 0:1],
                    yt[:, t:t + 1],
                    op0=ALU.subtract, op1=ALU.mult)
                nc.vector.tensor_tensor(t2[:, :], t2[:, :], g_ap,
                                        op=ALU.mult)
                nc.vector.tensor_tensor(out_tile[:, t, :], t2[:, :], b_ap,
                                        op=ALU.add)

    def transpose_group(self, ps_pool, src_fn, t_count, f_count, out_tile,
                        ps_bufs=2):
        nc = self.nc
        for f in range(f_count):
            ps = ps_pool.tile([P, 2048], FP16, tag="s",
                              name="ps_tr", bufs=2)
            for t in range(t_count):
                nc.tensor.transpose(ps[:, t * P:(t + 1) * P], src_fn(t, f),
                                    self.ident[:, :])
            if f % 2 == 0:
                nc.scalar.copy(out_tile[:, f, :], ps[:, 0:t_count * P])
            else:
                nc.vector.tensor_copy(out_tile[:, f, :],
                                      ps[:, 0:t_count * P])

    def mm(self, ps_ap, lhs_fn, rhs_fn, k_count):
        nc = self.nc
        for k in range(k_count):
            nc.tensor.matmul(ps_ap, lhs_fn(k), rhs_fn(k),
                             start=(k == 0), stop=(k == k_count - 1))

    # ---- attention part 1 (one elem): x-transpose + token-major V --------
    def attn_p1(self, T, x1, ps, act1, act2, qkvW_sb, zero_bias, vb):
        nc = self.nc
        N = T * P
        xt = act2.tile([P, DT, N], FP16, tag="ln_t", name="x1t")
        self.transpose_group(
            ps, lambda t, f: x1[:, t, f * P:(f + 1) * P],
            T, DT, xt, ps_bufs=2)
        v_sb = act2.tile([P, T, NH, HD + 4], FP16, tag="v_sb",
                         name="v_sb", bufs=2)
        nc.vector.memset(v_sb[:, :, :, 0:4], 1.0)
        for t in range(T):
            ps_v = ps.tile([P, D], FP32, tag="mm", name="ps_v", bufs=2)
            self.mm(
                ps_v[:, :],
                lambda k, t=t: xt[:, k, t * P:(t + 1) * P],
                lambda k: qkvW_sb[:, k, 2 * D:3 * D], DT)
            src = ps_v[:, :].rearrange("p (h f) -> p h f", f=HD)
            if not zero_bias:
                nc.vector.tensor_tensor(
                    v_sb[:, t, :, 4:], src,
                    vb[:, :].rearrange("p (h f) -> p h f", f=HD),
                    op=ALU.add)
            elif t % 2 == 0:
                nc.vector.tensor_copy(v_sb[:, t, :, 4:], src)
            else:
                nc.scalar.copy(v_sb[:, t, :, 4:], src)
        o_sb = act1.tile([P, T, D], FP16, tag="o_sb", name="o_sb", bufs=2)
        return xt, v_sb, o_sb

    # ---- attention qk GEMMs for one head pair of one elem ----------------
    def attn_qk(self, T, hp, xt, ps, act2, qkvW_sb, qkvb_cols, zero_bias):
        nc = self.nc
        N = T * P
        qkv3 = act2.tile([P, 2, N], FP16, tag="qkv3", name="qkv3",
                         bufs=2)
        if T == TE and zero_bias:
            ps_qk = ps.tile([P, 2, N], FP32, tag="mm", name="ps_qk",
                            bufs=2)
            for j, mi in enumerate((hp, 4 + hp)):
                self.mm(
                    ps_qk[:, j, :],
                    lambda k, mi=mi: qkvW_sb[:, k, mi * P:(mi + 1) * P],
                    lambda k: xt[:, k, :], DT)
            nc.vector.tensor_copy(qkv3[:, :, :], ps_qk[:, :, :])
        else:
            for j, mi in enumerate((hp, 4 + hp)):
                psq = ps.tile([P, N], FP32, tag="mm", name="ps_qk",
                                bufs=2)
                self.mm(
                    psq[:, :],
                    lambda k, mi=mi: qkvW_sb[:, k, mi * P:(mi + 1) * P],
                    lambda k: xt[:, k, :], DT)
                if zero_bias:
                    if j % 2 == 0:
                        nc.vector.tensor_copy(qkv3[:, j, :], psq[:, :])
                    else:
                        nc.scalar.copy(qkv3[:, j, :], psq[:, :])
                elif j % 2 == 0:
                    nc.vector.tensor_scalar(
                        qkv3[:, j, :], psq[:, :],
                        qkvb_cols[:, mi:mi + 1], None, op0=ALU.add)
                else:
                    nc.scalar.activation(
                        qkv3[:, j, :], psq[:, :], AF.Identity,
                        bias=qkvb_cols[:, mi:mi + 1])
        return qkv3

    # ---- attention head strand (one head, one elem) ----------------------
    def attn_head(self, T, hh, h_idx, qkv3, v_sb, o_sb, ps, act1,
                  a_b, coef_cols):
        nc = self.nc
        N = T * P
        base = hh * HD
        qa = qkv3[base:base + HD, 0, :]
        ka = qkv3[base:base + HD, 1, :]
        pt = act1.tile([P, T, N], FP16, tag="pT", name="pt", bufs=2)
        if T == TE:
            ps_s = ps.tile([P, 4, N], FP32, tag="s", name="ps_s",
                           bufs=2)
            for kk in range(T):
                nc.tensor.matmul(
                    ps_s[:, kk, :],
                    ka[:, kk * P:(kk + 1) * P], qa,
                    start=True, stop=True)
            s2 = self.mid.tile([P, T, N], FP16, tag="s2", name="s2")
            nc.vector.scalar_tensor_tensor(
                s2[:, :, :], a_b[:, :, :],
                coef_cols[:, h_idx:h_idx + 1], ps_s[:, 0:T, :],
                op0=ALU.mult, op1=ALU.add)
            nc.scalar.activation(pt[:, :, :], s2[:, :, :], AF.Exp)
        else:
            for kkh in range(T // 2):
                ps_s = ps.tile([P, 2, N], FP32, tag="s", name="ps_s",
                               bufs=2)
                for kk2 in range(2):
                    kk = 2 * kkh + kk2
                    nc.tensor.matmul(
                        ps_s[:, kk2, :],
                        ka[:, kk * P:(kk + 1) * P], qa,
                        start=True, stop=True)
                nc.scalar.activation(
                    pt[:, 2 * kkh:2 * kkh + 2, :],
                    ps_s[:, :, :], AF.Exp)
        # [rowsum | o] = pT.T @ [1|v]; one batched reciprocal per head
        ps_o = ps.tile([P, T, HD + 4], FP32, tag="o", name="ps_o", bufs=2)
        for m in range(T):
            for kk in range(T):
                nc.tensor.matmul(
                    ps_o[:, m, :],
                    pt[:, kk, m * P:(m + 1) * P],
                    v_sb[:, kk, h_idx, :],
                    start=(kk == 0), stop=(kk == T - 1))
        rinv = self.small.tile([P, T, 1], FP32, tag="rinv", name="rinv")
        nc.vector.reciprocal(rinv[:, :, :], ps_o[:, :, 0:1])
        for m in range(T):
            if m % 2 == 0:
                nc.vector.tensor_scalar(
                    o_sb[:, m, h_idx * HD:(h_idx + 1) * HD],
                    ps_o[:, m, 4:HD + 4], rinv[:, m, :],
                    None, op0=ALU.mult)
            else:
                nc.scalar.activation(
                    o_sb[:, m, h_idx * HD:(h_idx + 1) * HD],
                    ps_o[:, m, 4:HD + 4], AF.Copy,
                    scale=rinv[:, m, :])

    # ---- attention part 3 (one elem): o-transpose + proj + residual ------
    def attn_p3(self, T, h, o_sb, ps, act1, projW_sb, projb, zero_bias):
        nc = self.nc
        N = T * P
        ot = act1.tile([P, DT, N], FP16, tag="oT", name="ot", bufs=1)
        self.transpose_group(
            ps, lambda t, f: o_sb[:, t, f * P:(f + 1) * P],
            T, DT, ot, ps_bufs=2)
        for m in range(T):
            psp = ps.tile([P, D], FP32, tag="mm", name="ps_proj", bufs=2)
            self.mm(psp[:, :],
                    lambda k: ot[:, k, m * P:(m + 1) * P],
                    lambda k: projW_sb[:, k, :], DT)
            nc.vector.tensor_tensor(h[:, m, :], h[:, m, :], psp[:, :],
                                    op=ALU.add)
            if not zero_bias:
                nc.vector.tensor_tensor(h[:, m, :], h[:, m, :], projb,
                                        op=ALU.add)

    # ---- FFN (one elem) ---------------------------------------------------
    def ffn_body(self, T, h, x2, ps, act1, act2, f1W_sb, f2W_sb,
                 f1b_cols, f2b, zero_bias=False):
        nc = self.nc
        N = T * P
        xt = act2.tile([P, DT, N], FP16, tag="ln_t", name="x2t")
        self.transpose_group(
            ps, lambda t, f: x2[:, t, f * P:(f + 1) * P],
            T, DT, xt, ps_bufs=2)
        # f2 accumulators live in the score-tag PSUM slots (2 rows each)
        ps_f2 = [ps.tile([P, 2, D], FP32, tag="s", name=f"facc{mp}",
                         bufs=2)
                 for mp in range(T // 2)]
        half = FFT // 4
        for wave in range(4):
            gt = act1.tile([P, half, N], FP16, tag="gT", name="gt",
                           bufs=2)
            if T == TE and zero_bias:
                for jh in range(half // 2):
                    psf = ps.tile([P, 2, N], FP32, tag="mm", name="ps_f1",
                                   bufs=2)
                    for j2 in range(2):
                        mf = wave * half + 2 * jh + j2
                        self.mm(
                            psf[:, j2, :],
                            lambda k, mf=mf:
                                f1W_sb[:, k, mf * P:(mf + 1) * P],
                            lambda k: xt[:, k, :], DT)
                    mf0 = wave * half + 2 * jh
                    nc.scalar.activation(gt[:, 2 * jh:2 * jh + 2, :],
                                         psf[:, :, :], AF.Gelu,
                                         bias=f1b_cols[:, mf0:mf0 + 1])
            else:
                for j in range(half):
                    mf = wave * half + j
                    psf = ps.tile([P, N], FP32, tag="mm", name="ps_f1",
                                   bufs=2)
                    self.mm(
                        psf[:, :],
                        lambda k, mf=mf: f1W_sb[:, k, mf * P:(mf + 1) * P],
                        lambda k: xt[:, k, :], DT)
                    nc.scalar.activation(gt[:, j, :], psf[:, :], AF.Gelu,
                                         bias=f1b_cols[:, mf:mf + 1])
            for m in range(T):
                for j in range(half):
                    mf = wave * half + j
                    nc.tensor.matmul(
                        ps_f2[m // 2][:, m % 2, :],
                        gt[:, j, m * P:(m + 1) * P],
                        f2W_sb[:, mf, :],
                        start=(mf == 0), stop=(mf == FFT - 1))
        for m in range(T):
            nc.vector.tensor_tensor(h[:, m, :], h[:, m, :],
                                    ps_f2[m // 2][:, m % 2, :], op=ALU.add)
            if not zero_bias:
                nc.vector.tensor_tensor(h[:, m, :], h[:, m, :], f2b,
                                        op=ALU.add)

    # ---- model ----------------------------------------------------------
    def run(self, x_in, ab_in, ipW, qkvW, projW, f1W, f2W, up1W, up2W,
            rqkvW, rprojW, rf1W, rf2W, decW, ebc, epp, gbc, gpp, out_d):
        nc = self.nc
        tc = self.tc
        triv_ip, triv_encn, zb_enc, zb_hr, reuse_x = self.flags

        const = self.pool("const", 1)
        persist = self.pool("persist", 1)
        self.small = self.pool("small", 4)
        self.mid = self.pool("mid", 2)

        ident32 = const.tile([P, P], FP32)
        make_identity(nc, ident32[:, :])
        self.ident = const.tile([P, P], FP16)
        nc.vector.tensor_copy(self.ident[:, :], ident32[:, :])
        self.one_i = const.tile([P, 1], I32)
        nc.vector.memset(self.one_i[:, :], 1)
        self.magic_i = const.tile([P, TH], I32)
        nc.vector.memset(self.magic_i[:, :], MAGIC)
        self.warm_i = const.tile([P, 1], FP32)
        nc.vector.memset(self.warm_i[:, :], 0.0)
        self.warm_o = const.tile([P, 1], FP32)

        gpp_sb = persist.tile([P, 37], FP32)
        nc.sync.dma_start(out=gpp_sb[:, :], in_=gpp[:, :])

        hr_res = self.pool("hr_res", 1)
        h_hr = [hr_res.tile([P, TH, D], FP32, tag=f"Hhr{b}", name=f"Hhr{b}")
                for b in range(BE)]
        up_w = self.pool("up_w", 1)
        up_hf = self.pool("up_hf", 1)
        hfs = [up_hf.tile([P, TE, D], FP16, tag=f"hf{b}", name=f"hf{b}")
               for b in range(BE)]

        # ================= encoder region =================
        with pool_group(tc, [("enc_res", 1, "SBUF"), ("enc_a1", 1, "SBUF"),
                             ("enc_a2", 2, "SBUF"), ("enc_w", 1, "SBUF"),
                             ("enc_pk", 1, "SBUF"),
                             ("ps1", 1, "PSUM")]) as \
                (enc_res, act1, act2, enc_w, enc_pk, ps):
            h_enc = [enc_res.tile([P, TE, D], FP32, tag=f"Henc{b}",
                                  name=f"Henc{b}") for b in range(BE)]
            a_t = [enc_res.tile([P, TE, NLR], FP16, tag=f"A{b}", name=f"A{b}")
                   for b in range(BE)]
            for b in range(BE):
                for t in range(TE):
                    nc.scalar.dma_start(
                        out=a_t[b][:, t, :],
                        in_=ab_in[b].rearrange("(t p) m -> p t m",
                                               p=P)[:, t, :])

            def load_enc(l):
                w = {}
                w["qkv"] = enc_w.tile([P, DT, 3 * D], FP16, tag="qkvW",
                                      name="qkvW_sb", bufs=2)
                nc.sync.dma_start(
                    out=w["qkv"][:, :, :],
                    in_=qkvW[l].rearrange("(k p) n -> p k n", p=P))
                w["proj"] = enc_w.tile([P, DT, D], FP16, tag="projW",
                                       name="projW_sb", bufs=1)
                nc.sync.dma_start(
                    out=w["proj"][:, :, :],
                    in_=projW[l].rearrange("(k p) n -> p k n", p=P))
                w["f1"] = enc_w.tile([P, DT, FF], FP16, tag="f1W",
                                     name="f1W_sb", bufs=1)
                nc.sync.dma_start(
                    out=w["f1"][:, :, :],
                    in_=f1W[l].rearrange("(k p) n -> p k n", p=P))
                w["f2"] = enc_w.tile([P, FFT, D], FP16, tag="f2W",
                                     name="f2W_sb", bufs=1)
                nc.sync.dma_start(
                    out=w["f2"][:, :, :],
                    in_=f2W[l].rearrange("(k p) n -> p k n", p=P))
                w["ebc"] = enc_pk.tile([P, 3, D], FP32, tag="ebc",
                                       name="ebc_sb", bufs=1)
                nc.sync.dma_start(
                    out=w["ebc"][:, :, :],
                    in_=_bcast(ebc[l].rearrange("(a b) -> a b", b=D)))
                w["epp"] = enc_pk.tile([P, 36], FP32, tag="epp",
                                       name="epp_sb", bufs=2)
                nc.sync.dma_start(out=w["epp"][:, :], in_=epp[l])
                return w

            cur = load_enc(0)

            # ---------------- phase 0: input projection ----------------
            with pool_group(tc, [("ip_sb", 1, "SBUF"),
                                 ("ip_w", 1, "SBUF")]) as (ip_sb, ip_w):
                gbc_ip = ip_w.tile([P, 3, D], FP32)
                nc.scalar.dma_start(
                    out=gbc_ip[:, :, :],
                    in_=_bcast(gbc[0:3 * D].rearrange("(a b) -> a b", b=D)))
                ipW_sb = ip_w.tile([P, TE, D], FP16)
                nc.scalar.dma_start(
                    out=ipW_sb[:, :, :],
                    in_=ipW[:, :].rearrange("(k p) n -> p k n", p=P))
                x_sbs = []
                for b in range(BE):
                    if reuse_x:   # X_lr == sym(A_lr): reuse the A tiles
                        x_sbs.append(a_t[b])
                    else:
                        x_sb = ip_sb.tile([P, TE, NLR], FP16, tag=f"x{b}",
                                          name=f"x{b}")
                        nc.scalar.dma_start(
                            out=x_sb[:, :, :],
                            in_=x_in[b].rearrange("(t p) m -> p t m", p=P))
                        x_sbs.append(x_sb)
                # both elems' transposes+GEMMs first (b0's PSUM in "mm"
                # slots, b1's in an "s" slot) so neither waits on the
                # other's LN; then the LN/gelu chains.
                src_fns = []
                for b in range(BE):
                    xt = ip_sb.tile([P, TE, NLR], FP16, tag="xt", name="xt",
                                    bufs=2)
                    self.transpose_group(
                        ps,
                        lambda t, f, b=b: x_sbs[b][:, t, f * P:(f + 1) * P],
                        TE, TE, xt)
                    if b == 0:
                        zts = []
                        for m in range(TE):
                            zps = ps.tile([P, D], FP32, tag="mm", name="zps",
                                          bufs=2)
                            self.mm(zps[:, :],
                                    lambda k: xt[:, k, m * P:(m + 1) * P],
                                    lambda k: ipW_sb[:, k, :], TE)
                            zts.append(zps)
                        src_fns.append(lambda t, zts=zts: zts[t][:, :])
                    else:
                        zs = ps.tile([P, TE, D], FP32, tag="s", name="zps1",
                                     bufs=2)
                        for m in range(TE):
                            self.mm(zs[:, m, :],
                                    lambda k: xt[:, k, m * P:(m + 1) * P],
                                    lambda k: ipW_sb[:, k, :], TE)
                        src_fns.append(lambda t, zs=zs: zs[:, t, :])
                for b in range(BE):
                    src_fn = src_fns[b]
                    if not triv_ip:
                        z2 = ip_sb.tile([P, TE, D], FP32, tag=f"z{b}",
                                        name=f"z{b}")
                        for m in range(TE):
                            nc.vector.tensor_tensor(z2[:, m, :],
                                                    src_fn(m),
                                                    gbc_ip[:, 0, :],
                                                    op=ALU.add)
                        src_fn = lambda t, z2=z2: z2[:, t, :]
                    lno = ip_sb.tile([P, TE, D], FP32, tag=f"lnout{b}",
                                     name=f"lnout{b}")
                    self.ln_b(src_fn, TE, lno,
                              None if triv_ip else gbc_ip[:, 1, :],
                              None if triv_ip else gbc_ip[:, 2, :])
                    for t in range(TE):
                        nc.scalar.activation(h_enc[b][:, t, :],
                                             lno[:, t, :], AF.Gelu)
                self.warm_act(AF.Exp)

            # ---------------- encoder layers (pipelined) ----------------
            xs = [None, None]
            for b in range(BE):
                xs[b] = act2.tile([P, TE, D], FP16, tag="ln_out",
                                  name=f"xln{b}", bufs=2)
                self.ln_b(lambda t, b=b: h_enc[b][:, t, :], TE, xs[b])

            for l in range(L):
                w = cur
                if l + 1 < L:
                    cur = load_enc(l + 1)
                if l == L - 2:
                    up1W_sb = up_w.tile([P, TE, NHR], FP16)
                    nc.sync.dma_start(
                        out=up1W_sb[:, :, :],
                        in_=up1W[:, :].rearrange("(k p) n -> p k n", p=P))
                    up2W_sb = up_w.tile([P, TH, NHR], FP16)
                    nc.sync.dma_start(
                        out=up2W_sb[:, :, :],
                        in_=up2W[:, :].rearrange("(k p) n -> p k n", p=P))
                    if not triv_encn:
                        gbc_en = up_w.tile([P, 2, D], FP32)
                        nc.sync.dma_start(
                            out=gbc_en[:, :, :],
                            in_=_bcast(gbc[3 * D:5 * D].rearrange(
                                "(a b) -> a b", b=D)))

                st = [self.attn_p1(TE, xs[b], ps, act1, act2, w["qkv"],
                                   zb_enc, w["ebc"][:, 2, :])
                      for b in range(BE)]
                for hp in range(NH // 2):
                    qk = [self.attn_qk(TE, hp, st[b][0], ps, act2,
                                       w["qkv"], w["epp"][:, 0:12], zb_enc)
                          for b in range(BE)]
                    for hh in range(2):
                        for b in range(BE):
                            self.attn_head(
                                TE, hh, 2 * hp + hh, qk[b], st[b][1],
                                st[b][2], ps, act1, a_t[b],
                                w["epp"][:, 28:36])
                for b in range(BE):
                    self.attn_p3(TE, h_enc[b], st[b][2], ps, act1,
                                 w["proj"], w["ebc"][:, 0, :], zb_enc)
                    if b == 0:
                        self.warm_act(AF.Gelu)
                    xs[b] = act2.tile([P, TE, D], FP16, tag="ln_out",
                                      name=f"xln{b}", bufs=2)
                    self.ln_b(lambda t, b=b: h_enc[b][:, t, :], TE, xs[b])
                for b in range(BE):
                    self.ffn_body(
                        TE, h_enc[b], xs[b], ps, act1, act2,
                        w["f1"], w["f2"], w["epp"][:, 12:28],
                        w["ebc"][:, 1, :], zero_bias=zb_enc)
                    if b == 0 and l + 1 < L:
                        self.warm_act(AF.Exp)
                    if l + 1 < L:
                        xs[b] = act2.tile([P, TE, D], FP16, tag="ln_out",
                                          name=f"xln{b}", bufs=2)
                        self.ln_b(lambda t, b=b: h_enc[b][:, t, :], TE,
                                  xs[b])
                    else:
                        self.ln_b(
                            lambda t, b=b: h_enc[b][:, t, :], TE, hfs[b],
                            None if triv_encn else gbc_en[:, 0, :],
                            None if triv_encn else gbc_en[:, 1, :])

        # ================= upsample + HR + decoder region =================
        with pool_group(tc, [("hr_a1", 1, "SBUF"), ("hr_a2", 2, "SBUF"),
                             ("hr_w2", 1, "SBUF"), ("dec_sb", 1, "SBUF"),
                             ("dec_sb2", 2, "SBUF"),
                             ("ps2", 1, "PSUM")]) as \
                (act1, act2, hr_w2, dec_sb, dec_sb2, ps):
            rqkvW_sb = hr_w2.tile([P, DT, 3 * D], FP16, tag="qkvW")
            nc.sync.dma_start(
                out=rqkvW_sb[:, :, :],
                in_=rqkvW[:, :].rearrange("(k p) n -> p k n", p=P))
            gbc_hr = hr_w2.tile([P, 2, D], FP32, tag="gbc_hr")
            nc.sync.dma_start(
                out=gbc_hr[:, :, :],
                in_=_bcast(gbc[5 * D:7 * D].rearrange("(a b) -> a b", b=D)))
            if not zb_hr:
                vb_hr = hr_w2.tile([P, 1, D], FP32, tag="vb_hr")
                nc.sync.dma_start(
                    out=vb_hr[:, :, :],
                    in_=_bcast(gbc[9 * D:10 * D].rearrange(
                        "(a b) -> a b", b=D)))
            rprojW_sb = hr_w2.tile([P, DT, D], FP16, tag="projW")
            nc.sync.dma_start(
                out=rprojW_sb[:, :, :],
                in_=rprojW[:, :].rearrange("(k p) n -> p k n", p=P))
            rf1W_sb = hr_w2.tile([P, DT, FF], FP16, tag="f1W")
            nc.sync.dma_start(
                out=rf1W_sb[:, :, :],
                in_=rf1W[:, :].rearrange("(k p) n -> p k n", p=P))
            rf2W_sb = hr_w2.tile([P, FFT, D], FP16, tag="f2W")
            nc.sync.dma_start(
                out=rf2W_sb[:, :, :],
                in_=rf2W[:, :].rearrange("(k p) n -> p k n", p=P))
            decW_sb = hr_w2.tile([P, KDEC, DT, D], FP16, tag="decW")
            nc.sync.dma_start(
                out=decW_sb[:, :, :, :],
                in_=decW[:, :, :].rearrange("kd (k p) m -> p kd k m", p=P))

            rx = [None, None]
            for b in range(BE):
                g1 = act1.tile([P, TH, D], FP16, tag="g1", name="g1")
                for mh in range(TH):
                    psu = ps.tile([P, D], FP32, tag="mm", name="ps_up",
                             bufs=2)
                    self.mm(psu[:, :],
                            lambda k: up1W_sb[:, k, mh * P:(mh + 1) * P],
                            lambda k: hfs[b][:, k, :], TE)
                    nc.scalar.activation(g1[:, mh, :], psu[:, :], AF.Gelu,
                                         bias=gpp_sb[:, mh:mh + 1])
                for mh in range(TH):
                    psu = ps.tile([P, D], FP32, tag="mm", name="ps_up",
                             bufs=2)
                    self.mm(psu[:, :],
                            lambda k: up2W_sb[:, k, mh * P:(mh + 1) * P],
                            lambda k: g1[:, k, :], TH)
                    nc.vector.tensor_scalar(
                        h_hr[b][:, mh, :], psu[:, :],
                        gpp_sb[:, 4 + mh:5 + mh], None, op0=ALU.add)
                if b == 0:
                    self.warm_act(AF.Exp)
                rx[b] = act2.tile([P, TH, D], FP16, tag="ln_out",
                                  name=f"rx{b}", bufs=2)
                self.ln_b(lambda t, b=b: h_hr[b][:, t, :], TH, rx[b])

            st = [self.attn_p1(TH, rx[b], ps, act1, act2, rqkvW_sb,
                               zb_hr, None if zb_hr else vb_hr[:, 0, :])
                  for b in range(BE)]
            for hp in range(NH // 2):
                qk = [self.attn_qk(TH, hp, st[b][0], ps, act2,
                                   rqkvW_sb, gpp_sb[:, 8:20], zb_hr)
                      for b in range(BE)]
                for hh in range(2):
                    for b in range(BE):
                        self.attn_head(
                            TH, hh, 2 * hp + hh, qk[b], st[b][1],
                            st[b][2], ps, act1, None, None)
            for b in range(BE):
                self.attn_p3(TH, h_hr[b], st[b][2], ps, act1,
                             rprojW_sb, gbc_hr[:, 0, :], zb_hr)
                if b == 0:
                    self.warm_act(AF.Gelu)
                rx[b] = act2.tile([P, TH, D], FP16, tag="ln_out",
                                  name=f"rx{b}", bufs=2)
                self.ln_b(lambda t, b=b: h_hr[b][:, t, :], TH, rx[b])
            hf2s = [None, None]
            for b in range(BE):
                self.ffn_body(
                    TH, h_hr[b], rx[b], ps, act1, act2,
                    rf1W_sb, rf2W_sb, gpp_sb[:, 20:36],
                    gbc_hr[:, 1, :], zero_bias=zb_hr)
                if b == 0:
                    self.warm_act(AF.Ln)
                hf2s[b] = dec_sb2.tile([P, TH, D], FP16, tag="hf2",
                                       name=f"hf2{b}")
                self.ln_b(lambda t, b=b: h_hr[b][:, t, :], TH, hf2s[b])

            # ---------------- decoder ----------------
            # only one kd-slice of M1 = H W_kd^T is materialized at a time;
            # the bilinear row-block accumulators (2 PSUM "s" slots = 4 row
            # blocks) integrate over kd, and m1(kd+1) GEMMs cover ak(kd)'s
            # eviction latency.
            for b in range(BE):
                hft = dec_sb.tile([P, DT, NHR], FP16, tag="hft",
                                  name="hft", bufs=2)
                self.transpose_group(
                    ps, lambda t, f: hf2s[b][:, t, f * P:(f + 1) * P],
                    TH, DT, hft)
                psa = [ps.tile([P, 2, NHR], FP32, tag="s", name="ps_ak",
                               bufs=2) for _ in range(TH // 2)]
                m1ts = []

                def dec_m1(kd, hft=hft, b=b):
                    m1t = dec_sb.tile([P, DT, NHR], FP16, tag="m1t",
                                      name="m1t", bufs=2)
                    for mi in range(DT):
                        psd = ps.tile([P, NHR], FP32, tag="mm", name="ps_m1",
                                      bufs=2)
                        self.mm(
                            psd[:, :],
                            lambda k, kd=kd, mi=mi:
                                decW_sb[:, kd, k, mi * P:(mi + 1) * P],
                            lambda k: hft[:, k, :], DT)
                        if mi % 2 == 0:
                            nc.vector.tensor_copy(m1t[:, mi, :], psd[:, :])
                        else:
                            nc.scalar.copy(m1t[:, mi, :], psd[:, :])
                    return m1t

                def dec_ak(kd, m1t, hft=hft):
                    for md in range(TH):
                        wcol = NHR - md * P   # upper-tri column blocks
                        for k in range(DT):
                            nc.tensor.matmul(
                                psa[md // 2][:, md % 2, 0:wcol],
                                m1t[:, k, md * P:(md + 1) * P],
                                hft[:, k, md * P:],
                                start=(kd == 0 and k == 0),
                                stop=(kd == KDEC - 1 and k == DT - 1))

                m1ts.append(dec_m1(0))
                for kd in range(1, KDEC):
                    m1ts.append(dec_m1(kd))
                    dec_ak(kd - 1, m1ts[kd - 1])
                dec_ak(KDEC - 1, m1ts[KDEC - 1])
                out_sb = dec_sb2.tile([P, TH, NHR], FP16, tag="out",
                                      name="out_sb", bufs=1)
                for md in range(TH):
                    wcol = NHR - md * P
                    # softplus(x/K + b) = ln(1 + exp(x/K + b))
                    sp_e = self.mid.tile([P, NHR], FP32, tag="sp_e",
                                         name="sp_e")
                    nc.scalar.activation(sp_e[:, 0:wcol],
                                         psa[md // 2][:, md % 2, 0:wcol],
                                         AF.Exp,
                                         bias=gpp_sb[:, 36:37],
                                         scale=1.0 / KDEC)
                    nc.scalar.activation(out_sb[:, md, md * P:],
                                         sp_e[:, 0:wcol],
                                         AF.Ln, bias=1.0)
                    nc.sync.dma_start(
                        out=out_d[b].rearrange("(t p) m -> p t m",
                                               p=P)[:, md, md * P:],
                        in_=out_sb[:, md, md * P:])


# --------------------------------------------------------------------------
# host-side driver
# --------------------------------------------------------------------------
_CACHE = {}
_TRIU = np.triu_indices(NHR, k=1)


def _np(x):
    return np.ascontiguousarray(np.asarray(x, dtype=np.float32))


def _h(x):
    return np.ascontiguousarray(np.asarray(x).astype(np.float16))


def kernel(**inputs):
    res = run_on_device(inputs)
    full = np.concatenate([res.results[c]["OUT"] for c in range(NCORES)],
                          axis=0)  # (16, 512, 512)
    return np.ascontiguousarray(full[:, _TRIU[0], _TRIU[1]]).astype(np.float32)


def _fold_ln(g, b, w, bias):
    """(xn*g + b) @ w + bias  ==  xn @ (diag(g)w) + (bias + b @ w)."""
    w64 = w.astype(np.float64)
    w2 = (g.astype(np.float64)[:, None] * w64).astype(np.float32)
    b2 = (bias.astype(np.float64) + b.astype(np.float64) @ w64).astype(
        np.float32)
    return w2, b2


def run_on_device(inputs, **run_kwargs):
    inp = {k: _np(v) for k, v in inputs.items()}

    qkvW_f = np.empty_like(inp["e_qkvW"])
    qkvb_f = np.empty_like(inp["e_qkvb"])
    f1W_f = np.empty_like(inp["e_f1W"])
    f1b_f = np.empty_like(inp["e_f1b"])
    for l in range(L):
        qkvW_f[l], qkvb_f[l] = _fold_ln(inp["e_n1g"][l], inp["e_n1b"][l],
                                        inp["e_qkvW"][l], inp["e_qkvb"][l])
        f1W_f[l], f1b_f[l] = _fold_ln(inp["e_n2g"][l], inp["e_n2b"][l],
                                      inp["e_f1W"][l], inp["e_f1b"][l])
    rqkvW_f, rqkvb_f = _fold_ln(inp["r_n1g"], inp["r_n1b"],
                                inp["r_qkvW"], inp["r_qkvb"])
    rf1W_f, rf1b_f = _fold_ln(inp["r_n2g"], inp["r_n2b"],
                              inp["r_f1W"], inp["r_f1b"])
    # fold hd^-0.5 into the q columns (weights AND biases)
    qkvW_f[:, :, 0:D] *= HD ** -0.5
    qkvb_f[:, 0:D] *= HD ** -0.5
    rqkvW_f[:, 0:D] *= HD ** -0.5
    rqkvb_f[0:D] *= HD ** -0.5
    # fold the pre-decoder LN gain into the (symmetrized) decoder weights;
    # its bias is zero for this model (asserted).
    assert np.abs(inp["hrn_b"]).max() == 0.0, "hrn_b fold requires zero bias"
    dec_sym = 0.5 * (inp["dec_W"] + inp["dec_W"].transpose(0, 2, 1))
    g = inp["hrn_g"].astype(np.float64)
    dec_sym = (g[None, :, None] * dec_sym.astype(np.float64)
               * g[None, None, :]).astype(np.float32)

    # the transposed-score path uses A^T == A; guarantee symmetry
    a_sym = 0.5 * (inp["A_lr"] + inp["A_lr"].transpose(0, 2, 1))

    flags = (
        bool((inp["ip_g"] == 1).all() and (inp["ip_bt"] == 0).all()
             and (inp["ip_b"] == 0).all()),
        bool((inp["encn_g"] == 1).all() and (inp["encn_b"] == 0).all()),
        bool((inp["e_projb"] == 0).all() and (inp["e_f2b"] == 0).all()
             and (qkvb_f == 0).all() and (f1b_f == 0).all()),
        bool((inp["r_projb"] == 0).all() and (inp["r_f2b"] == 0).all()
             and (rqkvb_f == 0).all() and (rf1b_f == 0).all()),
        bool(np.array_equal(inp["X_lr"], a_sym)),
    )
    if _CACHE.get("flags") != flags:
        _CACHE["nc"] = build_nc(flags)
        _CACHE["flags"] = flags
    nc = _CACHE["nc"]

    ebc = np.stack([
        np.concatenate([inp["e_projb"][l], inp["e_f2b"][l],
                        qkvb_f[l, 2 * D:3 * D]])
        for l in range(L)
    ])
    epp = np.stack([
        np.concatenate([
            qkvb_f[l].reshape(12, P).T,
            f1b_f[l].reshape(FFT, P).T,
            np.broadcast_to(inp["e_ebs"][l] * inp["e_ebW"][l], (P, NH)),
        ], axis=1)
        for l in range(L)
    ])
    gbc = np.concatenate([
        inp["ip_b"], inp["ip_g"], inp["ip_bt"], inp["encn_g"], inp["encn_b"],
        inp["r_projb"], inp["r_f2b"], inp["hrn_g"], inp["hrn_b"],
        rqkvb_f[2 * D:3 * D],
    ])
    gpp = np.concatenate([
        inp["up1b"].reshape(TH, P).T,
        inp["up2b"].reshape(TH, P).T,
        rqkvb_f.reshape(12, P).T,
        rf1b_f.reshape(FFT, P).T,
        np.broadcast_to(inp["dec_b"][0], (P, 1)),
    ], axis=1)

    shared = {
        "ipW": _h(inp["ip_W"]), "qkvW": _h(qkvW_f), "projW": _h(inp["e_projW"]),
        "f1W": _h(f1W_f), "f2W": _h(inp["e_f2W"]), "up1W": _h(inp["up1W"]),
        "up2W": _h(inp["up2W"]), "rqkvW": _h(rqkvW_f),
        "rprojW": _h(inp["r_projW"]),
        "rf1W": _h(rf1W_f), "rf2W": _h(inp["r_f2W"]),
        "decW": _h(dec_sym),
        "ebc": np.ascontiguousarray(ebc), "epp": np.ascontiguousarray(epp),
        "gbc": np.ascontiguousarray(gbc), "gpp": np.ascontiguousarray(gpp),
    }
    in_maps = []
    for c in range(NCORES):
        m = dict(shared)
        m["X"] = _h(inp["X_lr"][c * BE:(c + 1) * BE])
        m["AB"] = _h(a_sym[c * BE:(c + 1) * BE])
        in_maps.append(m)

    return run_bass_kernel_spmd(nc, in_maps, list(range(NCORES)), **run_kwargs)


if __name__ == "__main__":
    import time
    t0 = time.time()
    nc = build_nc()
    print(f"build+finalize: {time.time() - t0:.1f}s, insts={len(nc.inst_map)}")
